# revision 20
# baseline (speedup 1.0000x reference)
"""NoPE attention block (QKV proj -> causal attention -> dense) on 8 TRN2 cores.

Sharding: tensor-parallel over heads. Each of the 8 cores computes 2 of the 16
heads end-to-end (its slice of the QKV projection, full causal attention for
those heads, and the corresponding 256 rows of the dense projection), producing
a partial [4096, 2048] output. The host sums the 8 partials and adds b_dense.

Math shortcuts vs the reference (exact, not approximations):
 - K bias dropped: scores (q+bq)@(k+bk) differ from (q+bq)@k only by terms
   constant over k, which softmax cancels.
 - V bias folded past the softmax AND the dense matmul: attention weights sum
   to 1, so the v-bias contributes bv @ W_dense — a constant row added on the
   host together with b_dense (see host_bias()).

Self-contained: only needs numpy/ml_dtypes/jax/concourse (all on PYTHONPATH).
"""

import numpy as np
import ml_dtypes

B, S, H, NH = 2, 2048, 2048, 16
HD = 128
NCORES = 8
NL = NH // NCORES            # heads per core = 2
T = B * S                    # 4096
SCALE = float(HD) ** -0.5

_BF16 = ml_dtypes.bfloat16

_CACHE = {}


def _build_nc(reps=1, h=H, s=S, b=B, phases=("qkv", "attn", "dense")):
    """Build + compile the per-core Bass program.

    reps>1 wraps the whole body in a hardware For_i loop (for slope timing).
    h/s/b can be shrunk for fast simulator tests.
    """
    import concourse.bass as bass  # noqa: F401
    import concourse.mybir as mybir
    import concourse.tile as tile
    from concourse import bacc
    from concourse.bass import ts, ds
    from concourse.masks import make_identity, make_upper_triangular

    f32 = mybir.dt.float32
    bf16 = mybir.dt.bfloat16
    FT = mybir.ActivationFunctionType
    ET = mybir.EngineType

    t_tot = b * s
    KS = h // 128            # contraction subtiles
    TC = 512                 # t-chunk for QKV
    NTC = t_tot // TC
    TT_N = t_tot // 128      # t tiles overall
    NCOL = max(1, h // 512)  # dense output column chunks

    nc = bacc.Bacc("TRN2", target_bir_lowering=False, debug=False)

    dynamic = reps == "dynamic"
    nreps = (
        nc.dram_tensor("nreps", [1, 1], mybir.dt.int32, kind="ExternalInput").ap()
        if dynamic
        else None
    )
    hid_t = nc.dram_tensor("hid_t", [h, t_tot], bf16, kind="ExternalInput").ap()
    wqk = nc.dram_tensor("wqk", [h, 2 * NL * HD], bf16, kind="ExternalInput").ap()
    wv = nc.dram_tensor("wv", [h, NL * HD], bf16, kind="ExternalInput").ap()
    wd = nc.dram_tensor("wd", [NL * HD, h], bf16, kind="ExternalInput").ap()
    bq = nc.dram_tensor("bq", [HD, NL], f32, kind="ExternalInput").ap()
    out = nc.dram_tensor("out_partial", [t_tot, h], bf16, kind="ExternalOutput").ap()

    hid_r = hid_t.rearrange("(ho p) t -> p ho t", p=128)
    wqk_r = wqk.rearrange("(ho p) j -> p ho j", p=128)
    wv_r = wv.rearrange("(ho p) j -> p ho j", p=128)
    wd_r = wd.rearrange("(ho p) n -> p ho n", p=128)
    out_r = out.rearrange("(to p) n -> p to n", p=128)

    with tile.TileContext(nc) as tc:
        with (
            tc.tile_pool(name="const", bufs=1) as const,
            tc.tile_pool(name="hid", bufs=4) as hidp,
            tc.tile_pool(name="work", bufs=3) as work,
            tc.tile_pool(name="psum", bufs=2, space="PSUM") as psum,
        ):
            # ---- constants (loaded once, outside the reps loop) ----
            wqk_sb = const.tile([128, KS, 2 * NL * HD], bf16, tag="wqk")
            wv_sb = const.tile([128, KS, NL * HD], bf16, tag="wv")
            pw0 = min(2, KS)
            # only the first wqk piece up-front; the rest go after the first
            # hidden piece so the first matmul can start ASAP
            nc.sync.dma_start(wqk_sb[:, ds(0, pw0), :], wqk_r[:, ds(0, pw0), :])
            wd_sb = const.tile([128, NL, h], bf16, tag="wd")
            bq_sb = const.tile([128, NL], f32, tag="bq")
            nc.sync.dma_start(bq_sb[:], bq)
            # triangle mask: tri[k, q] = 1 if k <= q else 0 (for diagonal tiles)
            tri_sb = const.tile([128, 128], bf16, tag="tri")
            make_upper_triangular(nc, tri_sb[:], val=1.0, diag=True)
            ident_sb = const.tile([128, 128], bf16, tag="ident")
            make_identity(nc, ident_sb[:])

            QW = 512
            QC_N = s // QW

            # persistent per-rep intermediates (fixed addresses; reps
            # serialize via the loop back-edge anyway)
            qkT_sb = const.tile([128, 2 * NL, t_tot], bf16, tag="qkT")
            v_sb = const.tile([128, TT_N, NL, HD + 1], bf16, tag="v")
            attnT_sb = const.tile([128, NL, t_tot], bf16, tag="attnT")
            # ones column for fused softmax denominator (never overwritten)
            nc.vector.memset(v_sb[:, :, :, HD : HD + 1], 1.0)
            # phase-ablation builds: init tensors the disabled phase would write
            if "qkv" not in phases:
                nc.vector.memset(qkT_sb[:], 0.0)
                nc.vector.memset(v_sb[:, :, :, 0:HD], 0.0)
            if "attn" not in phases and "dense" in phases:
                nc.vector.memset(attnT_sb[:], 0.0)

            hsm = KS >= 8  # hs-major first chunk (DMA-stream-rate consumption)
            c0_bounds = [0, 2, 6, 11, KS] if hsm else [0, KS]

            # chunk 0 lives in a persistent buffer: cold-loaded here (piece-
            # interleaved with the weights so the hs-major first pass can
            # consume at the serial DMA ring's stream rate), then RE-loaded
            # from inside phase C each rep so the next rep's QKV starts with
            # its data already resident (nothing can prefetch across the
            # For_i back-edge barrier).
            ht0_sb = const.tile([128, KS, TC], bf16, tag="ht0")
            for i in range(len(c0_bounds) - 1):
                p0, p1 = c0_bounds[i], c0_bounds[i + 1]
                nc.sync.dma_start(
                    ht0_sb[:, ds(p0, p1 - p0), :],
                    hid_r[:, ds(p0, p1 - p0), ts(0, TC)],
                )
                w0, w1 = c0_bounds[i + 1], c0_bounds[min(i + 2, len(c0_bounds) - 1)]
                if i == 0 and pw0 < c0_bounds[1]:
                    w0 = pw0
                if w1 > w0:
                    nc.sync.dma_start(
                        wqk_sb[:, ds(w0, w1 - w0), :],
                        wqk_r[:, ds(w0, w1 - w0), :],
                    )
            for w0, w1 in [(0, KS // 2), (KS // 2, KS)]:
                if w1 > w0:
                    nc.sync.dma_start(
                        wv_sb[:, ds(w0, w1 - w0), :],
                        wv_r[:, ds(w0, w1 - w0), :],
                    )

            def load_chunk(tci):
                ht = hidp.tile([128, KS, TC], bf16, tag="ht")
                q = max(1, KS // 4)
                bounds = sorted(set([0, q, 2 * q, 3 * q, KS]))
                for i in range(len(bounds) - 1):
                    p0, p1 = bounds[i], bounds[i + 1]
                    nc.sync.dma_start(
                        ht[:, ds(p0, p1 - p0), :],
                        hid_r[:, ds(p0, p1 - p0), ts(tci, TC)],
                    )
                return ht

            def compute_chunk(tci, ht):
                late = tci == NTC - 1  # runs in the ACT-loaded dense phase
                copy_ = nc.vector.tensor_copy if late else nc.scalar.copy

                def qk_drain(j, ps):
                    if j < NL:
                        # q: fused bias add on drain
                        nc.vector.tensor_scalar_add(
                            qkT_sb[:, j, ts(tci, TC)], ps[:], bq_sb[:, j : j + 1]
                        )
                    else:
                        # k: no bias needed (softmax-invariant)
                        copy_(qkT_sb[:, j, ts(tci, TC)], ps[:])

                # q^T / k^T:  psum[j, t] = sum_h W[h, j] * hidden^T[h, t]
                if tci == 0 and hsm:
                    # hs-major: all four j accumulators open at once so each
                    # DMA'd hs piece is consumed immediately
                    tgs = [("a", 2), ("a", 2), ("pv", 4), ("pv", 4)]
                    pss = [
                        psum.tile([128, TC], f32, tag=tg, bufs=nb, name=f"c0_{j}")
                        for j, (tg, nb) in enumerate(tgs)
                    ]
                    for hs in range(KS):
                        for j in range(2 * NL):
                            nc.tensor.matmul(
                                pss[j][:],
                                lhsT=wqk_sb[:, hs, ts(j, HD)],
                                rhs=ht[:, hs, :],
                                start=(hs == 0),
                                stop=(hs == KS - 1),
                            )
                    for j in range(2 * NL):
                        qk_drain(j, pss[j])
                else:
                    for j in range(2 * NL):
                        ps = psum.tile([128, TC], f32, tag="a", bufs=2)
                        for hs in range(KS):
                            nc.tensor.matmul(
                                ps[:],
                                lhsT=wqk_sb[:, hs, ts(j, HD)],
                                rhs=ht[:, hs, :],
                                start=(hs == 0),
                                stop=(hs == KS - 1),
                            )
                        qk_drain(j, ps)
                # v:  psum[t, j] = sum_h hidden^T[h, t] * Wv[h, j]  (no bias;
                # bv is folded into the attnT drain)
                vtag = ("s", 2) if tci == 0 and hsm else ("a", 2)
                for tt in range(TC // 128):
                    ps = psum.tile([128, NL, HD], f32, tag=vtag[0], bufs=vtag[1])
                    for hs in range(KS):
                        nc.tensor.matmul(
                            ps[:],
                            lhsT=ht[:, hs, ts(tt, 128)],
                            rhs=wv_sb[:, hs, :],
                            start=(hs == 0),
                            stop=(hs == KS - 1),
                        )
                    ti = tci * (TC // 128) + tt
                    copy_(v_sb[:, ti, :, 0:HD], ps[:])

            def attn_qc(bb, qc):
                for hh in range(NL):
                    # two paired accumulators, each one PSUM bank holding two
                    # q-tiles ([o~ | denom] x 2); 4 banks = 2 groups in flight
                    pvs = [
                        psum.tile(
                            [128, 2 * (HD + 1)], f32, tag="pv", bufs=4,
                            name=f"pv{p}",
                        )
                        for p in range(2)
                    ]
                    n_kt = 4 * qc + 4
                    for kt in range(n_kt):
                        d_ = kt - 4 * qc
                        q0 = max(d_, 0) * 128  # skip fully-masked q cols
                        sp = psum.tile([128, QW], f32, tag="s", bufs=2)
                        nc.tensor.matmul(
                            sp[:, q0:QW],
                            lhsT=qkT_sb[:, NL + hh, ds(bb * s + kt * 128, 128)],
                            rhs=qkT_sb[:, hh, ds(bb * s + qc * QW + q0, QW - q0)],
                            start=True,
                            stop=True,
                        )
                        e = work.tile([128, QW], bf16, tag="e", bufs=8)
                        nc.scalar.activation(
                            e[:, q0:QW], sp[:, q0:QW], FT.Exp, scale=SCALE
                        )
                        if d_ >= 0:
                            nc.vector.tensor_tensor(
                                e[:, ds(q0, 128)],
                                e[:, ds(q0, 128)],
                                tri_sb[:],
                                mybir.AluOpType.mult,
                            )
                        kg = (bb * s) // 128 + kt
                        for j in range(max(d_, 0), 4):
                            qt = 4 * qc + j
                            p, jj = divmod(j, 2)
                            # one start per bank clears its has_written bits;
                            # the pair partner then overwrites its own cleared
                            # columns and accumulates afterwards
                            nc.tensor.matmul(
                                pvs[p][:, jj * (HD + 1) : (jj + 1) * (HD + 1)],
                                lhsT=e[:, ts(j, 128)],
                                rhs=v_sb[:, kg, hh, :],
                                start=(kt == 0 and jj == 0),
                                stop=(kt == qt and jj == 1),
                            )
                    # drain: normalize, transpose all 4 q-tiles into one
                    # bank, then a single copy into attnT
                    tp = psum.tile([128, QW], bf16, tag="s", bufs=2, name="tp")
                    for j in range(4):
                        p, jj = divmod(j, 2)
                        base = jj * (HD + 1)
                        rec = work.tile([128, 1], f32, tag="rec", bufs=6)
                        nc.vector.reciprocal(rec[:], pvs[p][:, base + HD : base + HD + 1])
                        a_sb = work.tile([128, 128], bf16, tag="attn", bufs=6)
                        nc.vector.tensor_scalar_mul(
                            a_sb[:], pvs[p][:, base : base + HD], rec[:]
                        )
                        nc.tensor.transpose(tp[:, ts(j, 128)], a_sb[:], ident_sb[:])
                    nc.vector.tensor_copy(
                        attnT_sb[:, hh, ds(bb * s + qc * QW, QW)], tp[:]
                    )

            NW = min(512, h)

            def dense_tiles(tts, deep=False):
                # deep=True: attention psum tags are free (tail region) —
                # rotate across them for a deeper psum pipeline
                tags = ["a", "pv", "s", "pv"] if deep else ["a"]
                tagbufs = {"a": 2, "pv": 4, "s": 2}
                for gi, tt in enumerate(tts):
                    for ncc in range(NCOL):
                        tg = tags[(gi * NCOL + ncc) % len(tags)]
                        dps = psum.tile(
                            [128, NW], f32, tag=tg, bufs=tagbufs[tg], name="dps"
                        )
                        for hh in range(NL):
                            nc.tensor.matmul(
                                dps[:],
                                lhsT=attnT_sb[:, hh, ts(tt, 128)],
                                rhs=wd_sb[:, hh, ts(ncc, NW)],
                                start=(hh == 0),
                                stop=(hh == NL - 1),
                            )
                        dst = work.tile([128, NW], bf16, tag="dst", bufs=6)
                        # alternate drain engines; ACT also carries the exp
                        # stream, so DVE takes the larger share
                        if (gi * NCOL + ncc) % 3 == 2:
                            nc.scalar.copy(dst[:], dps[:])
                        else:
                            nc.vector.tensor_copy(dst[:], dps[:])
                        nc.sync.dma_start(out_r[:, tt, ts(ncc, NW)], dst[:])

            # Interleaved emission so ACT-bound attention overlaps PE-bound
            # QKV/dense work (the scheduler's lookahead is bounded, so the
            # interleave must happen at emission order).  npc == QC_N: one
            # QKV chunk produced per attention q-chunk consumed.
            npc = NTC // b  # qkv chunks per batch
            tpb = TT_N // b  # dense t-tiles per batch
            do_qkv = "qkv" in phases
            do_attn = "attn" in phases
            do_dense = "dense" in phases

            def body(_i):
                # phase A: batch-0 QKV, with one chunk of DMA prefetch
                if do_qkv:
                    hts = {0: ht0_sb}
                    for tci in range(npc):
                        if tci + 1 < npc:
                            hts[tci + 1] = load_chunk(tci + 1)
                        compute_chunk(tci, hts.pop(tci))
                        if tci == 0:
                            # deferred: keeps the startup DMA queue clear for
                            # the first hidden/weight pieces
                            nc.sync.dma_start(wd_sb[:], wd_r)
                # phase B: batch-0 attention interleaved with batch-1 QKV;
                # the last b1 chunk's compute is deferred to phase C as PE
                # filler there (its load still happens here)
                for qc in range(QC_N):
                    if do_qkv and b > 1 and qc < npc:
                        hts[npc + qc] = load_chunk(npc + qc)
                    if do_attn:
                        attn_qc(0, qc)
                    if do_qkv and b > 1 and qc < npc - 1:
                        compute_chunk(npc + qc, hts.pop(npc + qc))
                if b > 1:
                    # phase C: batch-1 attention + deferred last chunk + all
                    # dense tiles as PE filler
                    qtp = tpb // QC_N
                    for qc in range(QC_N):
                        if do_attn:
                            attn_qc(1, qc)
                        if do_qkv and qc == 0 and npc >= 1:
                            compute_chunk(NTC - 1, hts.pop(NTC - 1))
                            # next rep's chunk 0, loaded while the ring is slack
                            nc.sync.dma_start(ht0_sb[:], hid_r[:, :, ts(0, TC)])
                        if do_dense:
                            dense_tiles(range(qc * qtp, (qc + 1) * qtp))
                            if qc > 0:
                                dense_tiles(
                                    range(tpb + (qc - 1) * qtp, tpb + qc * qtp)
                                )
                    if do_dense:
                        dense_tiles(range(tpb + (QC_N - 1) * qtp, TT_N), deep=do_attn)
                elif do_dense:
                    dense_tiles(range(TT_N))

            hint = (ET.PE, ET.Activation, ET.DVE, ET.SP)
            if dynamic:
                nrep_sb = const.tile([1, 1], mybir.dt.int32, tag="nreps")
                nc.sync.dma_start(nrep_sb[:], nreps)
                rv = nc.sync.value_load(nrep_sb[:], min_val=1, max_val=100000)
                with tc.For_i(0, rv, 1, hint_engines=hint) as i:
                    body(i)
            elif reps == 1:
                body(0)
            else:
                with tc.For_i(0, reps, 1, hint_engines=hint) as i:
                    body(i)

    nc.compile()
    return nc


def _pack_inputs(hidden_states, W_qkv, b_qkv, W_dense):
    """Per-core input maps (host-side sharding)."""
    hid = np.asarray(hidden_states, dtype=np.float32).reshape(T, H)
    hid_t = np.ascontiguousarray(hid.T).astype(_BF16)
    W_qkv = np.asarray(W_qkv, dtype=np.float32)
    b_qkv = np.asarray(b_qkv, dtype=np.float32)
    W_dense = np.asarray(W_dense, dtype=np.float32)
    Wq, Wk, Wv = W_qkv[:, 0:H], W_qkv[:, H : 2 * H], W_qkv[:, 2 * H : 3 * H]
    bq = b_qkv[0:H]

    in_maps = []
    for c in range(NCORES):
        hs = [NL * c + i for i in range(NL)]
        cols = [Wq[:, h * HD : (h + 1) * HD] for h in hs] + [
            Wk[:, h * HD : (h + 1) * HD] for h in hs
        ]
        wqk_c = np.concatenate(cols, axis=1).astype(_BF16)
        wv_c = np.concatenate(
            [Wv[:, h * HD : (h + 1) * HD] for h in hs], axis=1
        ).astype(_BF16)
        wd_c = np.ascontiguousarray(
            W_dense[c * NL * HD : (c + 1) * NL * HD, :]
        ).astype(_BF16)
        bq_c = np.stack(
            [bq[h * HD : (h + 1) * HD] for h in hs], axis=1
        ).astype(np.float32)
        in_maps.append(
            {
                "hid_t": hid_t,
                "wqk": wqk_c,
                "wv": wv_c,
                "wd": wd_c,
                "bq": bq_c,
            }
        )
    return in_maps


def make_runner(nc, n_cores=NCORES):
    """Reusable jitted SPMD runner (no donation; device-resident inputs)."""
    import jax
    import concourse.mybir as mybir
    from jax.sharding import Mesh, PartitionSpec
    from jax.experimental.shard_map import shard_map
    from concourse.bass2jax import (
        _bass_exec_p,
        partition_id_tensor,
        install_neuronx_cc_hook,
    )

    install_neuronx_cc_hook()
    partition_name = nc.partition_id_tensor.name if nc.partition_id_tensor else None
    in_names, out_names, out_avals = [], [], []
    for alloc in nc.m.functions[0].allocations:
        if not isinstance(alloc, mybir.MemoryLocationSet):
            continue
        name = alloc.memorylocations[0].name
        if alloc.kind == "ExternalInput":
            if name != partition_name:
                in_names.append(name)
        elif alloc.kind == "ExternalOutput":
            out_names.append(name)
            out_avals.append(
                jax.core.ShapedArray(
                    tuple(alloc.tensor_shape), mybir.dt.np(alloc.dtype)
                )
            )
    n_params = len(in_names)
    all_in_names = list(in_names) + list(out_names)
    if partition_name is not None:
        all_in_names.append(partition_name)
    zero_outs = [np.zeros(a.shape, a.dtype) for a in out_avals]

    def _body(*args):
        operands = list(args)
        if partition_name is not None:
            operands.append(partition_id_tensor())
        outs = _bass_exec_p.bind(
            *operands,
            out_avals=tuple(out_avals),
            in_names=tuple(all_in_names),
            out_names=tuple(out_names),
            lowering_input_output_aliases=(),
            sim_require_finite=True,
            sim_require_nnan=True,
            nc=nc,
        )
        return tuple(outs)

    devices = jax.devices()[:n_cores]
    mesh = Mesh(np.asarray(devices), ("core",))
    in_specs = (PartitionSpec("core"),) * (n_params + len(out_names))
    out_specs = (PartitionSpec("core"),) * len(out_names)
    sharded = jax.jit(
        shard_map(
            _body, mesh=mesh, in_specs=in_specs, out_specs=out_specs, check_rep=False
        ),
        keep_unused=True,
    )

    def prepare(in_maps):
        per_core = [[np.asarray(m[name]) for name in in_names] for m in in_maps]
        concat_in = [
            np.concatenate([per_core[c][i] for c in range(n_cores)], axis=0)
            for i in range(n_params)
        ]
        concat_zero = [
            np.zeros((n_cores * z.shape[0], *z.shape[1:]), z.dtype) for z in zero_outs
        ]
        return [jax.device_put(a) for a in concat_in + concat_zero]

    def run(dev_args):
        outs = sharded(*dev_args)
        jax.block_until_ready(outs)
        return outs

    def fetch(outs):
        return [
            {
                name: np.asarray(outs[i]).reshape(n_cores, *out_avals[i].shape)[c]
                for i, name in enumerate(out_names)
            }
            for c in range(n_cores)
        ]

    return prepare, run, fetch


def host_bias(b_qkv, W_dense, b_dense):
    """Effective output bias: b_dense plus the folded v-bias term bv@W_dense."""
    bv = np.asarray(b_qkv, dtype=np.float64)[2 * H : 3 * H]
    return np.asarray(b_dense, dtype=np.float64) + bv @ np.asarray(
        W_dense, dtype=np.float64
    )


def kernel(hidden_states, W_qkv, b_qkv, W_dense, b_dense):
    from concourse import bass_utils

    if "nc1" not in _CACHE:
        _CACHE["nc1"] = _build_nc(reps=1)
    nc = _CACHE["nc1"]

    in_maps = _pack_inputs(hidden_states, W_qkv, b_qkv, W_dense)
    res = bass_utils.run_bass_kernel_spmd(nc, in_maps, core_ids=list(range(NCORES)))

    acc = np.zeros((T, H), dtype=np.float64)
    for c in range(NCORES):
        acc += res.results[c]["out_partial"].astype(np.float64)
    acc += host_bias(b_qkv, W_dense, b_dense)[None, :]
    return acc.astype(np.float32).reshape(B, S, H)
